# revision 1
# baseline (speedup 1.0000x reference)
"""Trainium2 Bass kernel for nn_AdaptiveMixedCoding (8 NeuronCores).

Sharding: data-parallel over B_img (8 images per core); caps/cap_lens/alpha
replicated. Caption Grams are computed split across cores (8 captions each)
and AllGathered.

Per-core algorithm (Bi=8 imgs, R=36 regions, Bc=64 caps, W=50 words, D=1024):
  S[row, c, w] = dot(imgs[row], caps[c, w])   one [288,1024]x[1024,3200] bf16
                 matmul; a K=1 ones-row accumulates bc_addS (0 valid / -1e6
                 masked) into the same PSUM group
  G[c]         = caps_c caps_c^T   (pair block-diag layout: G_{2p} at
                 [0:50,0:50], G_{2p+1} at [50:100,50:100] of Gp[:, p, :])
  t            = (S + bc_addS) * bc_scale'    (bc_scale' = inv_nc valid /
                 0.01 masked -> masked t ~= -1e4, finite)
  softmax: rowmax / exp(scale=10*inv_ni) / den;  hard = (t - rowmax == 0)
  mixed'       = hard + soft * a/((1-a) den)           (= mixed/(1-a))
  num'         = sum_w mixed' * S_sb
  qf'          = mixed'^T G mixed'   (pair transposes -> M_T,
                 u = M_T^T Gp in row space, flat products, reduce)
  out          = num'/(sqrt(qf') + eps/(1-a)), invalid img rows -> -1

End-to-end l2 rel err vs the f32 reference ~1.4e-3 (bf16 matmul rounding).
"""
import sys
import contextlib

sys.path.insert(0, '/opt/trn_rl_repo')

import numpy as np
import ml_dtypes

from concourse import bacc, tile, mybir

F32 = mybir.dt.float32
BF16 = mybir.dt.bfloat16
AF = mybir.ActivationFunctionType
OP = mybir.AluOpType
AX = mybir.AxisListType

N_CORES = 8
B, R, W, D = 64, 36, 50, 1024
BC = B
BI = B // N_CORES
ROWS = BI * R               # 288
CW = BC * W                 # 3200
KC = D // 128               # 8 contraction chunks
NP = BC // 2                # 32 caption pairs
PPC = NP // N_CORES         # 4 pairs (8 captions) per core
CPC = BC // N_CORES         # 8 captions per core
GW = CPC * W                # 400 caption-word columns per core
ROW_TILES = [(0, 108), (108, 108), (216, 72)]
N_CHUNKS = [(i * 512, min(512, CW - i * 512)) for i in range((CW + 511) // 512)]
EPS = 1e-8
NEGS = -1e6                 # pre-scale mask offset; *0.01 -> -1e4 in t
KMASK = 0.01
TINY = 1e-30

_CACHE = {}


def _build(a: float):
    am = max(a, 1e-6)
    oma = max(1.0 - a, 1e-6)

    nc = bacc.Bacc("TRN2", target_bir_lowering=False, debug=False,
                   num_devices=N_CORES)

    capsT = nc.declare_dram_parameter("capsT", [D, CW], BF16, isOutput=False)
    gcaps = nc.declare_dram_parameter("gcaps", [D, GW], BF16, isOutput=False)
    imgsT = nc.declare_dram_parameter("imgsT", [D, ROWS], BF16, isOutput=False)
    imgs_nat = nc.declare_dram_parameter("imgs_nat", [ROWS, D], F32,
                                         isOutput=False)
    scale_row_in = nc.declare_dram_parameter("scale_row_in", [1, CW], F32,
                                             isOutput=False)  # invnc*mask+off
    adds_row = nc.declare_dram_parameter("adds_row", [1, CW], F32,
                                         isOutput=False)    # 0 / NEGS
    iv_col = nc.declare_dram_parameter("iv_col", [ROWS, 1], F32,
                                       isOutput=False)
    ivm1_col = nc.declare_dram_parameter("ivm1_col", [ROWS, 1], F32,
                                         isOutput=False)
    out_ext = nc.declare_dram_parameter("out", [BI, BC, R], F32, isOutput=True)
    import os
    DEBUG = bool(os.environ.get("KERNEL_DEBUG"))
    if DEBUG:
        dbg_bc = nc.declare_dram_parameter("dbg_bc", [128, CW], F32,
                                           isOutput=True)
        dbg_nsq = nc.declare_dram_parameter("dbg_nsq", [128, NP], F32,
                                            isOutput=True)

    gb_in = nc.dram_tensor("gb_in", [PPC, 100, 128], BF16)
    gb_out = nc.dram_tensor("gb_out", [NP, 100, 128], BF16,
                            addr_space="Shared")

    with tile.TileContext(nc) as tc, contextlib.ExitStack() as ctx:
        const = ctx.enter_context(tc.tile_pool(name="const", bufs=1))
        big = ctx.enter_context(tc.tile_pool(name="big", bufs=1))
        work = ctx.enter_context(tc.tile_pool(name="work", bufs=2))
        small = ctx.enter_context(tc.tile_pool(name="small", bufs=2))
        workm = ctx.enter_context(tc.tile_pool(name="workm", bufs=3))
        psS = ctx.enter_context(tc.tile_pool(name="psS", bufs=2, space="PSUM"))
        psM = ctx.enter_context(tc.tile_pool(name="psM", bufs=6, space="PSUM"))

        # ---- constants --------------------------------------------------
        ident_bf = const.tile([128, 128], BF16)
        from concourse.masks import make_identity
        make_identity(nc, ident_bf[:])
        ident_f32 = const.tile([128, 128], F32)
        make_identity(nc, ident_f32[:])
        ones_bf = const.tile([1, 128], BF16)
        nc.gpsimd.memset(ones_bf[:], 1.0)

        # ---- input loads ------------------------------------------------
        caps_sb = big.tile([128, KC, CW], BF16)
        for kc in range(KC):
            nc.sync.dma_start(out=caps_sb[:, kc, :],
                              in_=capsT[kc * 128:(kc + 1) * 128, :])
        gcaps_sb = big.tile([128, KC, GW], BF16)
        nc.sync.dma_start(out=gcaps_sb[:],
                          in_=gcaps.rearrange("(k p) m -> p k m", p=128))
        imgsT_sb = big.tile([128, KC, ROWS], BF16)
        nc.sync.dma_start(out=imgsT_sb[:],
                          in_=imgsT.rearrange("(k p) m -> p k m", p=128))

        addsrow_sb = const.tile([1, CW], BF16)
        nc.gpsimd.dma_start(out=addsrow_sb[:], in_=adds_row[:])

        # ---- Grams for this core's 8 captions, then AllGather -----------
        # Gloc: even cap at [0:50, j, 0:50], odd cap at [64:114, j, 50:100]
        Gloc = big.tile([128, PPC, 128], BF16)
        nc.vector.memset(Gloc[:], 0.0)
        for lc in range(CPC):
            mw = min(128, GW - lc * W)   # FWL pad when possible
            gps = psM.tile([128, W], F32, tag="ps")
            for kc in range(KC):
                nc.tensor.matmul(gps[:mw, :],
                                 gcaps_sb[:, kc, lc * W:lc * W + mw],
                                 gcaps_sb[:, kc, lc * W:(lc + 1) * W],
                                 start=(kc == 0), stop=(kc == KC - 1))
            j, dd = divmod(lc, 2)
            if dd == 0:
                nc.scalar.activation(Gloc[0:W, j, 0:W], gps[0:W, :], AF.Copy)
            else:
                nc.scalar.activation(Gloc[64:64 + W, j, 50:100], gps[0:W, :],
                                     AF.Copy)
        # scale_row comes precomputed from the host (tiny DMA, bf16 cast)
        scale_row = workm.tile([1, CW], BF16, tag="mixed")
        nc.gpsimd.dma_start(out=scale_row[:], in_=scale_row_in[:])

        # full Gram gather (overlaps the S matmuls; needed only by qf)
        zb = const.tile([128, PPC * 100], BF16)
        nc.vector.memset(zb[:], 0.0)
        nc.gpsimd.dma_start(
            out=gb_in.rearrange("j r b -> (j r b)")[None, :],
            in_=zb[:])
        nc.gpsimd.dma_start(
            out=gb_in[:, 0:50, 0:50].rearrange("j r b -> r j b"),
            in_=Gloc[0:50, :, 0:50])
        nc.gpsimd.dma_start(
            out=gb_in[:, 50:100, 50:100].rearrange("j r b -> r j b"),
            in_=Gloc[64:114, :, 50:100])
        nc.gpsimd.collective_compute(
            "AllGather", OP.bypass,
            replica_groups=[list(range(N_CORES))],
            ins=[gb_in[:].opt()],
            outs=[gb_out[:].opt()],
        )
        # Gp[:, p, :]: G_{2p} at [0:50, 0:50], G_{2p+1} at [50:100, 50:100]
        Gp = big.tile([128, NP, 128], BF16)
        nc.vector.memset(Gp[:], 0.0)
        for k in range(N_CORES):
            nc.sync.dma_start(
                out=Gp[0:100, k * PPC:(k + 1) * PPC, :],
                in_=gb_out[k * PPC:(k + 1) * PPC, :, :].rearrange(
                    "j r b -> r j b"))

        # transposed mixed, pair-block layout (built per row tile)
        M_T = big.tile([128, NP, ROWS], BF16)
        nc.vector.memset(M_T[:, NP - 1, :], 0.0)

        # broadcast to 128 partitions via ones-matmul
        bc_scale = big.tile([128, CW], F32)
        for (n0, nw) in N_CHUNKS:
            bps = psM.tile([128, 512], F32, tag="ps")
            nc.tensor.matmul(bps[:, :nw], ones_bf[:],
                             scale_row[:, n0:n0 + nw], start=True, stop=True)
            nc.scalar.activation(bc_scale[:, n0:n0 + nw], bps[:, :nw], AF.Copy)

        if DEBUG:
            nc.scalar.dma_start(out=dbg_bc[:], in_=bc_scale[:])
            nc.scalar.dma_start(out=dbg_nsq[:], in_=invnc[:])

        # ---- per row-tile pipeline --------------------------------------
        for (r0, rt) in ROW_TILES:
            fwl = (r0 + 128 <= ROWS)
            mm = 128 if fwl else rt     # matmul M (junk rows not evacuated)
            img_nat_t = work.tile([128, D], F32, tag="imgnat")
            nc.sync.dma_start(out=img_nat_t[:rt, :],
                              in_=imgs_nat[r0:r0 + rt, :])
            sq_scr = work.tile([128, D], F32, tag="t")
            nsq_img = small.tile([128, 1], F32, tag="nsqimg")
            nc.scalar.activation(sq_scr[:rt, :], img_nat_t[:rt, :], AF.Square,
                                 accum_out=nsq_img[:rt, :])
            invni10 = small.tile([128, 1], F32, tag="invni10")
            nc.scalar.activation(invni10[:rt, :], nsq_img[:rt, :], AF.Sqrt,
                                 scale=0.01)
            nc.vector.reciprocal(invni10[:rt, :], invni10[:rt, :])
            iv_t = small.tile([128, 1], F32, tag="ivt")
            nc.gpsimd.dma_start(out=iv_t[:rt, :], in_=iv_col[r0:r0 + rt, :])
            ivm1_t = small.tile([128, 1], F32, tag="ivm1t")
            nc.gpsimd.dma_start(out=ivm1_t[:rt, :],
                                in_=ivm1_col[r0:r0 + rt, :])

            # S matmul (+ bias row) -> psum; evac raw S and masked-scaled t
            t = work.tile([128, CW], F32, tag="t")
            S_sb = work.tile([128, CW], F32, tag="S_sb")
            for (n0, nw) in N_CHUNKS:
                sps = psS.tile([128, 512], F32, tag="sps")
                for kc in range(KC):
                    nc.tensor.matmul(sps[:mm, :nw],
                                     imgsT_sb[:, kc, r0:r0 + mm],
                                     caps_sb[:, kc, n0:n0 + nw],
                                     start=(kc == 0), stop=False)
                nc.tensor.matmul(sps[:mm, :nw], ones_bf[:, :mm],
                                 addsrow_sb[:, n0:n0 + nw],
                                 start=False, stop=True)
                nc.scalar.activation(S_sb[:rt, n0:n0 + nw], sps[:rt, :nw],
                                     AF.Copy)
                # read from SBUF so the PSUM bank frees after the evac alone
                # (keeps the PE running ahead during the bc_scale prologue)
                nc.vector.tensor_tensor(t[:rt, n0:n0 + nw],
                                        S_sb[:rt, n0:n0 + nw],
                                        bc_scale[:rt, n0:n0 + nw], OP.mult)

            t3 = t[:rt, :].rearrange("p (c w) -> p c w", w=W)
            rowmax = small.tile([128, BC], F32, tag="rowmax")
            nc.vector.tensor_reduce(rowmax[:rt, :], t3, axis=AX.X, op=OP.max)
            nc.vector.tensor_tensor(
                t3, t3, rowmax[:rt, :, None].to_broadcast([rt, BC, W]),
                OP.subtract)
            exp_l = workm.tile([128, CW], BF16, tag="expl")
            nc.scalar.activation(exp_l[:rt, :], t[:rt, :], AF.Exp,
                                 scale=invni10[:rt, :])
            el3 = exp_l[:rt, :].rearrange("p (c w) -> p c w", w=W)
            den = small.tile([128, BC], F32, tag="den")
            nc.vector.tensor_reduce(den[:rt, :], el3, axis=AX.X, op=OP.add)
            invden = small.tile([128, BC], F32, tag="invden")
            nc.vector.tensor_scalar(invden[:rt, :], den[:rt, :], oma / am,
                                    oma * TINY / am, OP.mult, OP.add)
            nc.vector.reciprocal(invden[:rt, :], invden[:rt, :])
            nc.vector.tensor_tensor(
                el3, el3, invden[:rt, :, None].to_broadcast([rt, BC, W]),
                OP.mult)
            mixed = workm.tile([128, CW], BF16, tag="mixed")
            nc.vector.scalar_tensor_tensor(mixed[:rt, :], t[:rt, :], 0.0,
                                           exp_l[:rt, :], OP.is_equal, OP.add)

            # num' = sum_w mixed * S  (bf16 product into exp_l)
            nc.vector.tensor_tensor(exp_l[:rt, :], mixed[:rt, :], S_sb[:rt, :],
                                    OP.mult)
            num = small.tile([128, BC], F32, tag="num")
            nc.vector.tensor_reduce(
                num[:rt, :], el3, axis=AX.X, op=OP.add)

            # qf': single 128-wide transposes per pair, u in row space,
            # flat products into exp_l, one reduce
            for p in range(NP):
                c0 = 100 * p
                tw = min(128, CW - c0)
                tps = psM.tile([128, 128], BF16, tag="ps")
                nc.tensor.transpose(tps[0:tw, :rt],
                                    mixed[:rt, c0:c0 + tw],
                                    ident_bf[0:rt, 0:rt])
                nc.scalar.activation(M_T[0:tw, p, r0:r0 + rt], tps[0:tw, :rt],
                                     AF.Copy)
            for p in range(NP):
                ups = psM.tile([128, 128], F32, tag="ps")
                nc.tensor.matmul(ups[:rt, :], M_T[:, p, r0:r0 + rt],
                                 Gp[:, p, :], start=True, stop=True)
                nc.vector.tensor_tensor(exp_l[:rt, 100 * p:100 * p + 100],
                                        mixed[:rt, 100 * p:100 * p + 100],
                                        ups[:rt, 0:100], OP.mult)
            qf = small.tile([128, BC], F32, tag="qf")
            nc.vector.tensor_reduce(
                qf[:rt, :], el3, axis=AX.X, op=OP.add)

            # out = num/(sqrt(qf) + eps'); invalid rows -> -1
            denom = small.tile([128, BC], F32, tag="denom")
            nc.scalar.activation(denom[:rt, :], qf[:rt, :], AF.Sqrt)
            nc.vector.tensor_scalar(denom[:rt, :], denom[:rt, :], EPS / oma,
                                    None, OP.add)
            nc.vector.reciprocal(denom[:rt, :], denom[:rt, :])
            res = small.tile([128, BC], F32, tag="res")
            nc.vector.tensor_tensor(res[:rt, :], num[:rt, :], denom[:rt, :],
                                    OP.mult)
            nc.vector.tensor_scalar(res[:rt, :], res[:rt, :], iv_t[:rt, :],
                                    ivm1_t[:rt, :], OP.mult, OP.add)

            ops_ = psM.tile([BC, 128], F32, tag="ps")
            nc.tensor.transpose(ops_[:, :rt], res[:rt, :],
                                ident_f32[0:rt, 0:rt])
            out_sb = work.tile([BC, 128], F32, tag="imgnat")
            nc.scalar.activation(out_sb[:, :rt], ops_[:, :rt], AF.Copy)
            ni = rt // R
            i0 = r0 // R
            nc.scalar.dma_start(
                out=out_ext[i0:i0 + ni, :, :].rearrange("i c r -> c i r"),
                in_=out_sb[:, :rt].rearrange("c (i r) -> c i r", r=R))

    nc.finalize()
    return nc


def _get_runner(a: float):
    key = round(float(a), 9)
    if key not in _CACHE:
        _CACHE[key] = _build(key)
    return _CACHE[key]


def _host_prep(imgs, caps, img_lens, cap_lens):
    imgs = np.ascontiguousarray(np.asarray(imgs, dtype=np.float32))
    caps = np.ascontiguousarray(np.asarray(caps, dtype=np.float32))
    img_lens = np.asarray(img_lens).astype(np.int64)
    cap_lens = np.asarray(cap_lens).astype(np.int64)

    capsT = np.ascontiguousarray(
        caps.reshape(BC * W, D).T).astype(ml_dtypes.bfloat16)   # [D, CW]
    cap_mask = (np.arange(W)[:, None] < cap_lens[None, :]).astype(np.float32)
    adds_row = np.where(cap_mask.T.reshape(1, CW) > 0, 0.0,
                        NEGS).astype(np.float32)
    # pair-block masks [w~ 128, pair 32]; blocks at rows [0:50] / [50:100]
    inv_nc = 1.0 / (np.linalg.norm(caps.astype(np.float64), axis=-1) + EPS)
    cm_cw = cap_mask.T.reshape(1, CW)
    scale_row_in = (inv_nc.reshape(1, CW) * cm_cw
                    + KMASK * (1.0 - cm_cw)).astype(np.float32)

    in_maps = []
    for core in range(N_CORES):
        sl = slice(core * BI, (core + 1) * BI)
        im = imgs[sl].reshape(ROWS, D)
        imT = np.ascontiguousarray(im.T).astype(ml_dtypes.bfloat16)
        iv = (np.arange(R)[None, :] < img_lens[sl][:, None]).astype(
            np.float32).reshape(ROWS, 1)
        in_maps.append({
            "capsT": capsT,
            "gcaps": np.ascontiguousarray(capsT[:, core * GW:(core + 1) * GW]),
            "imgsT": imT,
            "imgs_nat": im,
            "scale_row_in": scale_row_in,
            "adds_row": adds_row,
            "iv_col": iv,
            "ivm1_col": iv - 1.0,
        })
    return in_maps


def run_on_device(inputs: dict, trace: bool = False):
    """Returns (output [64,64,36] f32, BassKernelResults)."""
    from concourse.bass_utils import run_bass_kernel_spmd
    alpha = float(np.asarray(inputs["alpha"]).reshape(-1)[0])
    a = 1.0 / (1.0 + np.exp(-alpha))
    nc = _get_runner(a)
    in_maps = _host_prep(inputs["imgs"], inputs["caps"], inputs["img_lens"],
                         inputs["cap_lens"])
    r = run_bass_kernel_spmd(nc, in_maps, list(range(N_CORES)), trace=trace)
    out = np.concatenate([r.results[c]["out"][None] for c in range(N_CORES)],
                         axis=0)
    return out.reshape(B, BC, R).astype(np.float32), r


def kernel(imgs, caps, img_lens, cap_lens, alpha):
    out, _ = run_on_device({"imgs": imgs, "caps": caps, "img_lens": img_lens,
                            "cap_lens": cap_lens, "alpha": alpha})
    return out



# revision 5
# speedup vs baseline: 1.0687x; 1.0687x over previous
"""Trainium2 Bass kernel for nn_AdaptiveMixedCoding (8 NeuronCores).

Sharding: data-parallel over B_img (8 images per core); caps/cap_lens/alpha
replicated. Caption Grams computed split across cores (8 caps each, batched
2-at-a-time) and AllGathered.

Per-core algorithm (Bi=8 imgs, R=36 regions, Bc=64 caps, W=50 words, D=1024):
  S[row, c, w] = dot(imgs[row], caps[c, w]) + adds   (fp16 matmul, K=1 ones
                 row accumulates 0 valid / -30000 masked into same PSUM group)
  t            = S_sb * bc_scale'  (fp16; bc_scale' = inv_nc valid / 0.01
                 masked -> masked t ~= -300, fp16-safe)
  exp          = Exp(t*invni10 + (-rowmax_all*invni10))   per-ROW max folded
                 into the scalar-engine bias (per-caption max cancels in the
                 softmax), so no per-caption subtract pass is needed
  hard         = (t == rowmax_c)   per-caption fp16 max, exact compare
  mixed'       = hard + exp * a/((1-a) den)            (= mixed/(1-a))
  num'         = sum_w mixed' * S_sb
  qf'          = mixed'^T G mixed'  (pair transposes -> M_T, u = M_T^T Gp,
                 4 pairs packed per PSUM bank, fused products, one reduce)
  out          = num'/(sqrt(qf') + eps/(1-a)), invalid img rows -> -1

Row tiles 128/128/32 (full partition use). DMA order: gcaps, imgsT, caps by
column-chunks, imgs_nat last -> grams + S matmuls start early; Gram
AllGather triggers ~10us in and overlaps the S phase.

End-to-end l2 rel err vs the f32 reference ~8e-3 (fp16 t argmax ties).
"""
import sys
import contextlib

sys.path.insert(0, '/opt/trn_rl_repo')

import numpy as np
import ml_dtypes

from concourse import bacc, tile, mybir

F32 = mybir.dt.float32
F16 = mybir.dt.float16
AF = mybir.ActivationFunctionType
OP = mybir.AluOpType
AX = mybir.AxisListType

N_CORES = 8
B, R, W, D = 64, 36, 50, 1024
BC = B
BI = B // N_CORES
ROWS = BI * R               # 288
CW = BC * W                 # 3200
KC = D // 128               # 8 contraction chunks
NP = BC // 2                # 32 caption pairs
PPC = NP // N_CORES         # 4 pairs (8 captions) per core
CPC = BC // N_CORES         # 8 captions per core
GW = PPC * 128              # padded gram columns per core (50|pad|50 per pair)
ROW_TILES = [(0, 128), (128, 128), (256, 32)]
N_CHUNKS = [(i * 512, min(512, CW - i * 512)) for i in range((CW + 511) // 512)]
EPS = 1e-8
NEGS = -30000.0             # masked S offset; fp16-safe, *0.01 -> t ~= -300
KMASK = 0.01
TINY = 1e-30

_CACHE = {}


def _build(a: float):
    am = max(a, 1e-6)
    oma = max(1.0 - a, 1e-6)

    nc = bacc.Bacc("TRN2", target_bir_lowering=False, debug=False,
                   num_devices=N_CORES)

    capsT = nc.declare_dram_parameter("capsT", [D, CW], F16, isOutput=False)
    gcaps = nc.declare_dram_parameter("gcaps", [D, GW], F16, isOutput=False)
    imgsT = nc.declare_dram_parameter("imgsT", [D, ROWS], F16, isOutput=False)
    imgs_nat = nc.declare_dram_parameter("imgs_nat", [ROWS, D], F32,
                                         isOutput=False)
    scale_row_in = nc.declare_dram_parameter("scale_row_in", [1, CW], F16,
                                             isOutput=False)  # invnc / KMASK
    adds_row = nc.declare_dram_parameter("adds_row", [1, CW], F16,
                                         isOutput=False)    # 0 / NEGS
    iv_col = nc.declare_dram_parameter("iv_col", [ROWS, 1], F32,
                                       isOutput=False)
    ivm1_col = nc.declare_dram_parameter("ivm1_col", [ROWS, 1], F32,
                                         isOutput=False)
    out_ext = nc.declare_dram_parameter("out", [BI, BC, R], F32, isOutput=True)

    gb_in = nc.dram_tensor("gb_in", [PPC, 100, 128], F16)
    gb_out = nc.dram_tensor("gb_out", [NP, 100, 128], F16,
                            addr_space="Shared")

    with tile.TileContext(nc) as tc, contextlib.ExitStack() as ctx:
        const = ctx.enter_context(tc.tile_pool(name="const", bufs=1))
        big = ctx.enter_context(tc.tile_pool(name="big", bufs=1))
        work = ctx.enter_context(tc.tile_pool(name="work", bufs=2))
        small = ctx.enter_context(tc.tile_pool(name="small", bufs=2))
        psS = ctx.enter_context(tc.tile_pool(name="psS", bufs=2, space="PSUM"))
        psQ = ctx.enter_context(tc.tile_pool(name="psQ", bufs=2, space="PSUM"))
        psM = ctx.enter_context(tc.tile_pool(name="psM", bufs=4, space="PSUM"))

        # ---- constants --------------------------------------------------
        ident_16 = const.tile([128, 128], F16)
        from concourse.masks import make_identity
        make_identity(nc, ident_16[:])
        ident_f32 = const.tile([128, 128], F32)
        make_identity(nc, ident_f32[:])
        ones_16 = const.tile([1, 128], F16)
        nc.gpsimd.memset(ones_16[:], 1.0)

        # ---- input loads (priority order) -------------------------------
        gcaps_sb = big.tile([128, KC, GW], F16)
        nc.sync.dma_start(out=gcaps_sb[:],
                          in_=gcaps.rearrange("(k p) m -> p k m", p=128))
        imgsT_sb = big.tile([128, KC, ROWS], F16)
        nc.sync.dma_start(out=imgsT_sb[:],
                          in_=imgsT.rearrange("(k p) m -> p k m", p=128))
        caps_sb = big.tile([128, KC, CW], F16)
        capsT_r = capsT.rearrange("(k p) m -> p k m", p=128)
        for (n0, nw) in N_CHUNKS:
            nc.sync.dma_start(out=caps_sb[:, :, n0:n0 + nw],
                              in_=capsT_r[:, :, n0:n0 + nw])

        addsrow_sb = const.tile([1, CW], F16)
        nc.gpsimd.dma_start(out=addsrow_sb[:], in_=adds_row[:])
        scale_row = const.tile([1, CW], F16)
        nc.gpsimd.dma_start(out=scale_row[:], in_=scale_row_in[:])

        # ---- Grams for this core's 8 captions (2 caps per matmul) -------
        # Gloc: even cap at [0:50, j, 0:50], odd cap at [64:114, j, 50:100]
        Gloc = big.tile([128, PPC, 128], F16)
        nc.vector.memset(Gloc[:], 0.0)
        for j in range(PPC):
            c0 = j * 128
            gps = psM.tile([128, 128], F32, tag="ps")
            for kc in range(KC):
                nc.tensor.matmul(gps[:, :],
                                 gcaps_sb[:, kc, c0:c0 + 128],
                                 gcaps_sb[:, kc, c0:c0 + 128],
                                 start=(kc == 0), stop=(kc == KC - 1))
            nc.scalar.activation(Gloc[0:50, j, 0:50], gps[0:50, 0:50],
                                 AF.Copy)
            nc.scalar.activation(Gloc[64:114, j, 50:100], gps[64:114, 64:114],
                                 AF.Copy)

        # full Gram gather (overlaps the S matmuls; needed only by qf)
        zb = const.tile([128, PPC * 100], F16)
        nc.vector.memset(zb[:], 0.0)
        nc.gpsimd.dma_start(
            out=gb_in.rearrange("j r b -> (j r b)")[None, :],
            in_=zb[:])
        nc.gpsimd.dma_start(
            out=gb_in[:, 0:50, 0:50].rearrange("j r b -> r j b"),
            in_=Gloc[0:50, :, 0:50])
        nc.gpsimd.dma_start(
            out=gb_in[:, 50:100, 50:100].rearrange("j r b -> r j b"),
            in_=Gloc[64:114, :, 50:100])
        nc.gpsimd.collective_compute(
            "AllGather", OP.bypass,
            replica_groups=[list(range(N_CORES))],
            ins=[gb_in[:].opt()],
            outs=[gb_out[:].opt()],
        )
        # Gp[:, p, :]: G_{2p} at [0:50, 0:50], G_{2p+1} at [50:100, 50:100]
        Gp = big.tile([128, NP, 128], F16)
        nc.vector.memset(Gp[:], 0.0)
        for k in range(N_CORES):
            nc.sync.dma_start(
                out=Gp[0:100, k * PPC:(k + 1) * PPC, :],
                in_=gb_out[k * PPC:(k + 1) * PPC, :, :].rearrange(
                    "j r b -> r j b"))

        # transposed mixed, pair-block layout (built per row tile)
        M_T = big.tile([128, NP, ROWS], F16)
        nc.vector.memset(M_T[:, NP - 1, :], 0.0)

        # broadcast scale_row to 128 partitions via ones-matmul
        bc_scale = big.tile([128, CW], F16)
        for (n0, nw) in N_CHUNKS:
            bps = psM.tile([128, 512], F32, tag="ps")
            nc.tensor.matmul(bps[:, :nw], ones_16[:],
                             scale_row[:, n0:n0 + nw], start=True, stop=True)
            nc.scalar.activation(bc_scale[:, n0:n0 + nw], bps[:, :nw], AF.Copy)

        # persistent output accumulator [BC, ROWS]
        out_sb = big.tile([BC, ROWS], F32)

        # ---- pipelined per-row-tile phases ------------------------------
        def s_phase(r0, rt):
            """S matmul + bias; evac fp16 S_sb; t = S_sb * bc_scale."""
            mm = 128 if (r0 + 128 <= ROWS) else rt
            img_nat_t = work.tile([128, D], F32, tag="imgnat")
            nc.sync.dma_start(out=img_nat_t[:rt, :],
                              in_=imgs_nat[r0:r0 + rt, :])
            sq_scr = work.tile([128, D], F32, tag="sqscr")
            nsq_img = small.tile([128, 1], F32, tag="nsqimg")
            nc.scalar.activation(sq_scr[:rt, :], img_nat_t[:rt, :], AF.Square,
                                 accum_out=nsq_img[:rt, :])
            invni10 = small.tile([128, 1], F32, tag="invni10")
            nc.scalar.activation(invni10[:rt, :], nsq_img[:rt, :], AF.Sqrt,
                                 scale=0.01)
            nc.vector.reciprocal(invni10[:rt, :], invni10[:rt, :])
            iv_t = small.tile([128, 1], F32, tag="ivt")
            nc.gpsimd.dma_start(out=iv_t[:rt, :], in_=iv_col[r0:r0 + rt, :])
            ivm1_t = small.tile([128, 1], F32, tag="ivm1t")
            nc.gpsimd.dma_start(out=ivm1_t[:rt, :],
                                in_=ivm1_col[r0:r0 + rt, :])

            t = work.tile([128, CW], F16, tag="t")
            S_sb = work.tile([128, CW], F16, tag="S_sb")
            for (n0, nw) in N_CHUNKS:
                sps = psS.tile([128, 512], F32, tag="sps")
                for kc in range(KC):
                    nc.tensor.matmul(sps[:mm, :nw],
                                     imgsT_sb[:, kc, r0:r0 + mm],
                                     caps_sb[:, kc, n0:n0 + nw],
                                     start=(kc == 0), stop=False)
                nc.tensor.matmul(sps[:mm, :nw], ones_16[:, :mm],
                                 addsrow_sb[:, n0:n0 + nw],
                                 start=False, stop=True)
                nc.scalar.activation(S_sb[:rt, n0:n0 + nw], sps[:rt, :nw],
                                     AF.Copy)
                nc.vector.tensor_tensor(t[:rt, n0:n0 + nw],
                                        S_sb[:rt, n0:n0 + nw],
                                        bc_scale[:rt, n0:n0 + nw], OP.mult)
            return t, S_sb, invni10, iv_t, ivm1_t

        def v_phase(r0, rt, t, S_sb, invni10):
            """softmax/hard/mixed + num (vector+scalar engines)."""
            t3 = t[:rt, :].rearrange("p (c w) -> p c w", w=W)
            rowmax = small.tile([128, BC], F16, tag="rowmax")
            nc.vector.tensor_reduce(rowmax[:rt, :], t3, axis=AX.X, op=OP.max)
            nrm_all = small.tile([128, 1], F32, tag="nrmall")
            nc.vector.tensor_reduce(nrm_all[:rt, :], rowmax[:rt, :],
                                    axis=AX.X, op=OP.max, negate=True)
            nbias = small.tile([128, 1], F32, tag="nbias")
            nc.vector.tensor_scalar(nbias[:rt, :], nrm_all[:rt, :],
                                    invni10[:rt, :], None, OP.mult)
            el = work.tile([128, CW], F16, tag="el")
            nc.scalar.activation(el[:rt, :], t[:rt, :], AF.Exp,
                                 bias=nbias[:rt, :], scale=invni10[:rt, :])
            el3 = el[:rt, :].rearrange("p (c w) -> p c w", w=W)
            den = small.tile([128, BC], F32, tag="den")
            nc.vector.tensor_reduce(den[:rt, :], el3, axis=AX.X, op=OP.add)
            invden = small.tile([128, BC], F32, tag="invden")
            nc.vector.tensor_scalar(invden[:rt, :], den[:rt, :], oma / am,
                                    oma * TINY / am, OP.mult, OP.add)
            nc.vector.reciprocal(invden[:rt, :], invden[:rt, :])
            soft = work.tile([128, CW], F16, tag="soft")
            s3 = soft[:rt, :].rearrange("p (c w) -> p c w", w=W)
            nc.vector.tensor_tensor(
                s3, el3, invden[:rt, :, None].to_broadcast([rt, BC, W]),
                OP.mult)
            mixed = work.tile([128, CW], F16, tag="mixed")
            m3 = mixed[:rt, :].rearrange("p (c w) -> p c w", w=W)
            # hard into el (dead after soft), then mixed = soft + hard
            nc.vector.tensor_tensor(
                el3, t3, rowmax[:rt, :, None].to_broadcast([rt, BC, W]),
                OP.is_equal)
            nc.vector.tensor_tensor(m3, s3, el3, OP.add)

            # num' = sum_w mixed * S  (prod into soft, dead now)
            nc.vector.tensor_tensor(soft[:rt, :], mixed[:rt, :], S_sb[:rt, :],
                                    OP.mult)
            num = small.tile([128, BC], F32, tag="num")
            nc.vector.tensor_reduce(num[:rt, :], s3, axis=AX.X, op=OP.add)
            return mixed, num

        def qf_phase(r0, rt, mixed, num, iv_t, ivm1_t):
            """qf' via Gram quadratic form; out row assembly."""
            for p in range(NP):
                c0 = 100 * p
                tw = min(128, CW - c0)
                tps = psM.tile([128, 128], F16, tag="ps")
                nc.tensor.transpose(tps[0:tw, :rt],
                                    mixed[:rt, c0:c0 + tw],
                                    ident_16[0:rt, 0:rt])
                nc.scalar.activation(M_T[0:tw, p, r0:r0 + rt], tps[0:tw, :rt],
                                     AF.Copy)
            qprod = work.tile([128, CW], F16, tag="el")  # el ring reuse
            for q in range(NP // 4):
                ups = psQ.tile([128, 512], F32, tag="ups")
                for pi in range(4):
                    p = 4 * q + pi
                    nc.tensor.matmul(ups[:rt, 128 * pi:128 * pi + 128],
                                     M_T[:, p, r0:r0 + rt],
                                     Gp[:, p, :], start=True, stop=True)
                u4 = ups[:rt, :].rearrange("p (j x) -> p j x", x=128)
                nc.vector.tensor_tensor(
                    qprod[:rt, 400 * q:400 * q + 400].rearrange(
                        "p (j w) -> p j w", w=100),
                    mixed[:rt, 400 * q:400 * q + 400].rearrange(
                        "p (j w) -> p j w", w=100),
                    u4[:, :, 0:100], OP.mult)
            qf = small.tile([128, BC], F32, tag="qf")
            nc.vector.tensor_reduce(
                qf[:rt, :],
                qprod[:rt, :].rearrange("p (c w) -> p c w", w=W),
                axis=AX.X, op=OP.add)

            # out = num/(sqrt(qf) + eps'); invalid rows -> -1
            denom = small.tile([128, BC], F32, tag="denom")
            nc.scalar.activation(denom[:rt, :], qf[:rt, :], AF.Sqrt)
            nc.vector.tensor_scalar(denom[:rt, :], denom[:rt, :], EPS / oma,
                                    None, OP.add)
            nc.vector.reciprocal(denom[:rt, :], denom[:rt, :])
            res = small.tile([128, BC], F32, tag="res")
            nc.vector.tensor_tensor(res[:rt, :], num[:rt, :], denom[:rt, :],
                                    OP.mult)
            nc.vector.tensor_scalar(res[:rt, :], res[:rt, :], iv_t[:rt, :],
                                    ivm1_t[:rt, :], OP.mult, OP.add)

            ops_ = psM.tile([BC, 128], F32, tag="ps")
            nc.tensor.transpose(ops_[:, :rt], res[:rt, :],
                                ident_f32[0:rt, 0:rt])
            nc.scalar.activation(out_sb[:, r0:r0 + rt], ops_[:, :rt], AF.Copy)

        # software pipeline: S(t0) S(t1) V(t0) QF(t0) S(t2) V(t1) QF(t1) ...
        st = {}
        st[0] = s_phase(*ROW_TILES[0])
        st[1] = s_phase(*ROW_TILES[1])
        m0, n0_ = v_phase(ROW_TILES[0][0], ROW_TILES[0][1], *st[0][:3])
        qf_phase(ROW_TILES[0][0], ROW_TILES[0][1], m0, n0_,
                 st[0][3], st[0][4])
        st[2] = s_phase(*ROW_TILES[2])
        m1, n1_ = v_phase(ROW_TILES[1][0], ROW_TILES[1][1], *st[1][:3])
        qf_phase(ROW_TILES[1][0], ROW_TILES[1][1], m1, n1_,
                 st[1][3], st[1][4])
        m2, n2_ = v_phase(ROW_TILES[2][0], ROW_TILES[2][1], *st[2][:3])
        qf_phase(ROW_TILES[2][0], ROW_TILES[2][1], m2, n2_,
                 st[2][3], st[2][4])

        # single final output DMA: out_sb [BC, ROWS] -> out_ext [BI, BC, R]
        nc.scalar.dma_start(
            out=out_ext.rearrange("i c r -> c i r"),
            in_=out_sb[:].rearrange("c (i r) -> c i r", r=R))

    nc.finalize()
    return nc


def _get_runner(a: float):
    key = round(float(a), 9)
    if key not in _CACHE:
        _CACHE[key] = _build(key)
    return _CACHE[key]


def _gcaps_padded(capsT, core):
    """[D, PPC*128]: pair j -> even cap words at cols 128j+0:50, odd cap
    words at 128j+64:114, rest zero (32-aligned PSUM block reads)."""
    g = np.zeros((D, GW), dtype=np.float16)
    base = core * CPC * W
    for j in range(PPC):
        g[:, 128 * j:128 * j + 50] = capsT[:, base + 100 * j:
                                           base + 100 * j + 50]
        g[:, 128 * j + 64:128 * j + 114] = capsT[:, base + 100 * j + 50:
                                                 base + 100 * j + 100]
    return g


def _host_prep(imgs, caps, img_lens, cap_lens):
    imgs = np.ascontiguousarray(np.asarray(imgs, dtype=np.float32))
    caps = np.ascontiguousarray(np.asarray(caps, dtype=np.float32))
    img_lens = np.asarray(img_lens).astype(np.int64)
    cap_lens = np.asarray(cap_lens).astype(np.int64)

    capsT = np.ascontiguousarray(
        caps.reshape(BC * W, D).T).astype(np.float16)   # [D, CW]
    cap_mask = (np.arange(W)[:, None] < cap_lens[None, :]).astype(np.float32)
    cm_cw = cap_mask.T.reshape(1, CW)
    adds_row = np.where(cm_cw > 0, 0.0, NEGS).astype(np.float16)
    inv_nc = 1.0 / (np.linalg.norm(caps.astype(np.float64), axis=-1) + EPS)
    scale_row_in = (inv_nc.reshape(1, CW) * cm_cw
                    + KMASK * (1.0 - cm_cw)).astype(np.float16)

    in_maps = []
    for core in range(N_CORES):
        sl = slice(core * BI, (core + 1) * BI)
        im = imgs[sl].reshape(ROWS, D)
        imT = np.ascontiguousarray(im.T).astype(np.float16)
        iv = (np.arange(R)[None, :] < img_lens[sl][:, None]).astype(
            np.float32).reshape(ROWS, 1)
        in_maps.append({
            "capsT": capsT,
            "gcaps": _gcaps_padded(capsT, core),
            "imgsT": imT,
            "imgs_nat": im,
            "scale_row_in": scale_row_in,
            "adds_row": adds_row,
            "iv_col": iv,
            "ivm1_col": iv - 1.0,
        })
    return in_maps


def run_on_device(inputs: dict, trace: bool = False):
    """Returns (output [64,64,36] f32, BassKernelResults)."""
    from concourse.bass_utils import run_bass_kernel_spmd
    alpha = float(np.asarray(inputs["alpha"]).reshape(-1)[0])
    a = 1.0 / (1.0 + np.exp(-alpha))
    nc = _get_runner(a)
    in_maps = _host_prep(inputs["imgs"], inputs["caps"], inputs["img_lens"],
                         inputs["cap_lens"])
    r = run_bass_kernel_spmd(nc, in_maps, list(range(N_CORES)), trace=trace)
    out = np.concatenate([r.results[c]["out"][None] for c in range(N_CORES)],
                         axis=0)
    return out.reshape(B, BC, R).astype(np.float32), r


def kernel(imgs, caps, img_lens, cap_lens, alpha):
    out, _ = run_on_device({"imgs": imgs, "caps": caps, "img_lens": img_lens,
                            "cap_lens": cap_lens, "alpha": alpha})
    return out


# revision 6
# speedup vs baseline: 1.2326x; 1.1534x over previous
"""Trainium2 Bass kernel for nn_AdaptiveMixedCoding (8 NeuronCores).

Sharding: data-parallel over B_img (8 images per core); caps/cap_lens/alpha
replicated. Caption Grams computed split across cores (8 caps each, batched
2-at-a-time) and AllGathered.

Per-core algorithm (Bi=8 imgs, R=36 regions, Bc=64 caps, W=50 words, D=1024):
  S[row, c, w] = dot(imgs[row], caps[c, w]) + adds   (fp16 matmul, K=1 ones
                 row accumulates 0 valid / -30000 masked into same PSUM group)
  t            = S_sb * bc_scale'  (fp16; bc_scale' = inv_nc valid / 0.01
                 masked -> masked t ~= -300, fp16-safe)
  exp          = Exp(t*invni10 + (-rowmax_all*invni10))   per-ROW max folded
                 into the scalar-engine bias (per-caption max cancels in the
                 softmax), so no per-caption subtract pass is needed
  hard         = (t == rowmax_c)   per-caption fp16 max, exact compare
  mixed'       = hard + exp * a/((1-a) den)            (= mixed/(1-a))
  num'         = sum_w mixed' * S_sb
  qf'          = mixed'^T G mixed'  (pair transposes -> M_T, u = M_T^T Gp,
                 4 pairs packed per PSUM bank, fused products, one reduce)
  out          = num'/(sqrt(qf') + eps/(1-a)), invalid img rows -> -1

Row tiles 128/128/32 (full partition use). DMA order: gcaps, imgsT, caps by
column-chunks, imgs_nat last -> grams + S matmuls start early; Gram
AllGather triggers ~10us in and overlaps the S phase.

End-to-end l2 rel err vs the f32 reference ~8e-3 (fp16 t argmax ties).
"""
import sys
import contextlib

sys.path.insert(0, '/opt/trn_rl_repo')

import numpy as np
import ml_dtypes

from concourse import bacc, tile, mybir

F32 = mybir.dt.float32
F16 = mybir.dt.float16
AF = mybir.ActivationFunctionType
OP = mybir.AluOpType
AX = mybir.AxisListType

N_CORES = 8
B, R, W, D = 64, 36, 50, 1024
BC = B
BI = B // N_CORES
ROWS = BI * R               # 288
CW = BC * W                 # 3200
KC = D // 128               # 8 contraction chunks
NP = BC // 2                # 32 caption pairs
PPC = NP // N_CORES         # 4 pairs (8 captions) per core
CPC = BC // N_CORES         # 8 captions per core
GW = PPC * 128              # padded gram columns per core (50|pad|50 per pair)
ROW_TILES = [(0, 128), (128, 128), (256, 32)]
N_CHUNKS = [(i * 512, min(512, CW - i * 512)) for i in range((CW + 511) // 512)]
EPS = 1e-8
NEGS = -30000.0             # masked S offset; fp16-safe, *0.01 -> t ~= -300
KMASK = 0.01
TINY = 1e-30

_CACHE = {}


def _build(a: float):
    am = max(a, 1e-6)
    oma = max(1.0 - a, 1e-6)

    nc = bacc.Bacc("TRN2", target_bir_lowering=False, debug=False,
                   num_devices=N_CORES)

    capsT = nc.declare_dram_parameter("capsT", [D, CW], F16, isOutput=False)
    gcaps = nc.declare_dram_parameter("gcaps", [D, GW], F16, isOutput=False)
    imgsT = nc.declare_dram_parameter("imgsT", [D, ROWS], F16, isOutput=False)
    imgs_nat = nc.declare_dram_parameter("imgs_nat", [ROWS, D], F32,
                                         isOutput=False)
    bc_scale_in = nc.declare_dram_parameter("bc_scale_in", [128, CW], F16,
                                            isOutput=False)  # invnc / KMASK
    adds_row = nc.declare_dram_parameter("adds_row", [1, CW], F16,
                                         isOutput=False)    # 0 / NEGS
    iv_col = nc.declare_dram_parameter("iv_col", [ROWS, 1], F32,
                                       isOutput=False)
    ivm1_col = nc.declare_dram_parameter("ivm1_col", [ROWS, 1], F32,
                                         isOutput=False)
    out_ext = nc.declare_dram_parameter("out", [BI, BC, R], F32, isOutput=True)

    gb_in = nc.dram_tensor("gb_in", [PPC, 100, 128], F16)
    gb_out = nc.dram_tensor("gb_out", [NP, 100, 128], F16,
                            addr_space="Shared")

    with tile.TileContext(nc) as tc, contextlib.ExitStack() as ctx:
        const = ctx.enter_context(tc.tile_pool(name="const", bufs=1))
        big = ctx.enter_context(tc.tile_pool(name="big", bufs=1))
        work = ctx.enter_context(tc.tile_pool(name="work", bufs=2))
        work3 = ctx.enter_context(tc.tile_pool(name="work3", bufs=3))
        scr = ctx.enter_context(tc.tile_pool(name="scr", bufs=1))
        small = ctx.enter_context(tc.tile_pool(name="small", bufs=2))
        psS = ctx.enter_context(tc.tile_pool(name="psS", bufs=2, space="PSUM"))
        psQ = ctx.enter_context(tc.tile_pool(name="psQ", bufs=2, space="PSUM"))
        psM = ctx.enter_context(tc.tile_pool(name="psM", bufs=2, space="PSUM"))
        psT = ctx.enter_context(tc.tile_pool(name="psT", bufs=2, space="PSUM"))

        # ---- constants --------------------------------------------------
        ident_16 = const.tile([128, 128], F16)
        from concourse.masks import make_identity
        make_identity(nc, ident_16[:])
        ident_f32 = const.tile([128, 128], F32)
        make_identity(nc, ident_f32[:])
        ones_16 = const.tile([1, 128], F16)
        nc.gpsimd.memset(ones_16[:], 1.0)

        # ---- input loads (priority order) -------------------------------
        gcaps_sb = big.tile([128, KC, GW], F16)
        nc.sync.dma_start(out=gcaps_sb[:],
                          in_=gcaps.rearrange("(k p) m -> p k m", p=128))
        imgsT_sb = big.tile([128, KC, ROWS], F16)
        nc.sync.dma_start(out=imgsT_sb[:],
                          in_=imgsT.rearrange("(k p) m -> p k m", p=128))
        caps_sb = big.tile([128, KC, CW], F16)
        capsT_r = capsT.rearrange("(k p) m -> p k m", p=128)
        for (n0, nw) in N_CHUNKS:
            nc.sync.dma_start(out=caps_sb[:, :, n0:n0 + nw],
                              in_=capsT_r[:, :, n0:n0 + nw])

        addsrow_sb = const.tile([1, CW], F16)
        nc.gpsimd.dma_start(out=addsrow_sb[:], in_=adds_row[:])

        # ---- Grams for this core's 8 captions (2 caps per matmul) -------
        # Gloc: even cap at [0:50, j, 0:50], odd cap at [64:114, j, 50:100]
        Gloc = big.tile([128, PPC, 128], F16)
        nc.vector.memset(Gloc[:], 0.0)
        for j in range(PPC):
            c0 = j * 128
            gps = psM.tile([128, 128], F32, tag="ps")
            for kc in range(KC):
                nc.tensor.matmul(gps[:, :],
                                 gcaps_sb[:, kc, c0:c0 + 128],
                                 gcaps_sb[:, kc, c0:c0 + 128],
                                 start=(kc == 0), stop=(kc == KC - 1))
            nc.scalar.activation(Gloc[0:50, j, 0:50], gps[0:50, 0:50],
                                 AF.Copy)
            nc.scalar.activation(Gloc[64:114, j, 50:100], gps[64:114, 64:114],
                                 AF.Copy)

        # full Gram gather (overlaps the S matmuls; needed only by qf)
        zb = const.tile([128, PPC * 100], F16)
        nc.vector.memset(zb[:], 0.0)
        nc.gpsimd.dma_start(
            out=gb_in.rearrange("j r b -> (j r b)")[None, :],
            in_=zb[:])
        nc.gpsimd.dma_start(
            out=gb_in[:, 0:50, 0:50].rearrange("j r b -> r j b"),
            in_=Gloc[0:50, :, 0:50])
        nc.gpsimd.dma_start(
            out=gb_in[:, 50:100, 50:100].rearrange("j r b -> r j b"),
            in_=Gloc[64:114, :, 50:100])
        nc.gpsimd.collective_compute(
            "AllGather", OP.bypass,
            replica_groups=[list(range(N_CORES))],
            ins=[gb_in[:].opt()],
            outs=[gb_out[:].opt()],
        )
        # Gp[:, p, :]: G_{2p} at [0:50, 0:50], G_{2p+1} at [50:100, 50:100]
        Gp = big.tile([128, NP, 128], F16)
        nc.vector.memset(Gp[:], 0.0)
        for k in range(N_CORES):
            nc.sync.dma_start(
                out=Gp[0:100, k * PPC:(k + 1) * PPC, :],
                in_=gb_out[k * PPC:(k + 1) * PPC, :, :].rearrange(
                    "j r b -> r j b"))

        # transposed mixed, pair-block layout (built per row tile)
        M_T = big.tile([128, NP, ROWS], F16)
        nc.vector.memset(M_T[:, NP - 1, :], 0.0)

        # bc_scale comes host-prebroadcast [128, CW]
        bc_scale = big.tile([128, CW], F16)
        nc.sync.dma_start(out=bc_scale[:], in_=bc_scale_in[:])

        # persistent output accumulator [BC, ROWS]
        out_sb = big.tile([BC, ROWS], F32)

        # ---- pipelined per-row-tile phases ------------------------------
        def s_phase(r0, rt):
            """S matmul + bias; evac fp16 S_sb; t = S_sb * bc_scale."""
            mm = 128 if (r0 + 128 <= ROWS) else rt
            img_nat_t = work.tile([128, D], F32, tag="imgnat")
            nc.sync.dma_start(out=img_nat_t[:rt, :],
                              in_=imgs_nat[r0:r0 + rt, :])
            sq_scr = scr.tile([128, D], F32, tag="sqscr")
            nsq_img = small.tile([128, 1], F32, tag="nsqimg")
            nc.scalar.activation(sq_scr[:rt, :], img_nat_t[:rt, :], AF.Square,
                                 accum_out=nsq_img[:rt, :])
            invni10 = small.tile([128, 1], F32, tag="invni10")
            nc.scalar.activation(invni10[:rt, :], nsq_img[:rt, :], AF.Sqrt,
                                 scale=0.01)
            nc.vector.reciprocal(invni10[:rt, :], invni10[:rt, :])
            iv_t = small.tile([128, 1], F32, tag="ivt")
            nc.gpsimd.dma_start(out=iv_t[:rt, :], in_=iv_col[r0:r0 + rt, :])
            ivm1_t = small.tile([128, 1], F32, tag="ivm1t")
            nc.gpsimd.dma_start(out=ivm1_t[:rt, :],
                                in_=ivm1_col[r0:r0 + rt, :])

            t = work3.tile([128, CW], F16, tag="t")
            S_sb = work3.tile([128, CW], F16, tag="S_sb")
            for (n0, nw) in N_CHUNKS:
                sps = psS.tile([128, 512], F32, tag="sps")
                for kc in range(KC):
                    nc.tensor.matmul(sps[:mm, :nw],
                                     imgsT_sb[:, kc, r0:r0 + mm],
                                     caps_sb[:, kc, n0:n0 + nw],
                                     start=(kc == 0), stop=False)
                nc.tensor.matmul(sps[:mm, :nw], ones_16[:, :mm],
                                 addsrow_sb[:, n0:n0 + nw],
                                 start=False, stop=True)
                nc.scalar.activation(S_sb[:rt, n0:n0 + nw], sps[:rt, :nw],
                                     AF.Copy)
                nc.vector.tensor_tensor(t[:rt, n0:n0 + nw],
                                        S_sb[:rt, n0:n0 + nw],
                                        bc_scale[:rt, n0:n0 + nw], OP.mult)
            return t, S_sb, invni10, iv_t, ivm1_t

        def v_phase(r0, rt, t, S_sb, invni10):
            """softmax/hard/mixed + num (vector+scalar engines)."""
            t3 = t[:rt, :].rearrange("p (c w) -> p c w", w=W)
            rowmax = small.tile([128, BC], F16, tag="rowmax")
            nc.vector.tensor_reduce(rowmax[:rt, :], t3, axis=AX.X, op=OP.max)
            nrm_all = small.tile([128, 1], F32, tag="nrmall")
            nc.vector.tensor_reduce(nrm_all[:rt, :], rowmax[:rt, :],
                                    axis=AX.X, op=OP.max, negate=True)
            nbias = small.tile([128, 1], F32, tag="nbias")
            nc.vector.tensor_scalar(nbias[:rt, :], nrm_all[:rt, :],
                                    invni10[:rt, :], None, OP.mult)
            el = work.tile([128, CW], F16, tag="el")
            nc.scalar.activation(el[:rt, :], t[:rt, :], AF.Exp,
                                 bias=nbias[:rt, :], scale=invni10[:rt, :])
            el3 = el[:rt, :].rearrange("p (c w) -> p c w", w=W)
            den = small.tile([128, BC], F32, tag="den")
            nc.vector.tensor_reduce(den[:rt, :], el3, axis=AX.X, op=OP.add)
            invden = small.tile([128, BC], F32, tag="invden")
            nc.vector.tensor_scalar(invden[:rt, :], den[:rt, :], oma / am,
                                    oma * TINY / am, OP.mult, OP.add)
            nc.vector.reciprocal(invden[:rt, :], invden[:rt, :])
            soft = work.tile([128, CW], F16, tag="soft")
            s3 = soft[:rt, :].rearrange("p (c w) -> p c w", w=W)
            nc.vector.tensor_tensor(
                s3, el3, invden[:rt, :, None].to_broadcast([rt, BC, W]),
                OP.mult)
            mixed = work.tile([128, CW], F16, tag="mixed")
            m3 = mixed[:rt, :].rearrange("p (c w) -> p c w", w=W)
            # hard into el (dead after soft), then mixed = soft + hard
            nc.vector.tensor_tensor(
                el3, t3, rowmax[:rt, :, None].to_broadcast([rt, BC, W]),
                OP.is_equal)
            nc.vector.tensor_tensor(m3, s3, el3, OP.add)

            # num' = sum_w mixed * S  (prod into soft, dead now)
            nc.vector.tensor_tensor(soft[:rt, :], mixed[:rt, :], S_sb[:rt, :],
                                    OP.mult)
            num = small.tile([128, BC], F32, tag="num")
            nc.vector.tensor_reduce(num[:rt, :], s3, axis=AX.X, op=OP.add)
            return mixed, num

        def qf_phase(r0, rt, mixed, num, iv_t, ivm1_t):
            """qf' via Gram quadratic form; out row assembly."""
            for q in range(NP // 4):
                tps = psT.tile([128, 512], F16, tag="tps")
                for pi in range(4):
                    p = 4 * q + pi
                    c0 = 100 * p
                    tw = min(128, CW - c0)
                    nc.tensor.transpose(tps[0:tw, 128 * pi:128 * pi + rt],
                                        mixed[:rt, c0:c0 + tw],
                                        ident_16[0:rt, 0:rt])
                t4 = tps[:, :].rearrange("p (j x) -> p j x", x=128)
                if q < 7:
                    nc.scalar.activation(
                        M_T[:, 4 * q:4 * q + 4, r0:r0 + rt],
                        t4[:, :, 0:rt], AF.Copy)
                else:
                    nc.scalar.activation(
                        M_T[:, 28:31, r0:r0 + rt],
                        t4[:, 0:3, 0:rt], AF.Copy)
                    nc.scalar.activation(
                        M_T[0:100, 31, r0:r0 + rt],
                        t4[0:100, 3, 0:rt], AF.Copy)
            qprod = work.tile([128, CW], F16, tag="el")  # el ring reuse
            for q in range(NP // 4):
                ups = psQ.tile([128, 512], F32, tag="ups")
                for pi in range(4):
                    p = 4 * q + pi
                    nc.tensor.matmul(ups[:rt, 128 * pi:128 * pi + 128],
                                     M_T[:, p, r0:r0 + rt],
                                     Gp[:, p, :], start=True, stop=True)
                u4 = ups[:rt, :].rearrange("p (j x) -> p j x", x=128)
                nc.vector.tensor_tensor(
                    qprod[:rt, 400 * q:400 * q + 400].rearrange(
                        "p (j w) -> p j w", w=100),
                    mixed[:rt, 400 * q:400 * q + 400].rearrange(
                        "p (j w) -> p j w", w=100),
                    u4[:, :, 0:100], OP.mult)
            qf = small.tile([128, BC], F32, tag="qf")
            nc.vector.tensor_reduce(
                qf[:rt, :],
                qprod[:rt, :].rearrange("p (c w) -> p c w", w=W),
                axis=AX.X, op=OP.add)

            # out = num/(sqrt(qf) + eps'); invalid rows -> -1
            denom = small.tile([128, BC], F32, tag="denom")
            nc.scalar.activation(denom[:rt, :], qf[:rt, :], AF.Sqrt)
            nc.vector.tensor_scalar(denom[:rt, :], denom[:rt, :], EPS / oma,
                                    None, OP.add)
            nc.vector.reciprocal(denom[:rt, :], denom[:rt, :])
            res = small.tile([128, BC], F32, tag="res")
            nc.vector.tensor_tensor(res[:rt, :], num[:rt, :], denom[:rt, :],
                                    OP.mult)
            nc.vector.tensor_scalar(res[:rt, :], res[:rt, :], iv_t[:rt, :],
                                    ivm1_t[:rt, :], OP.mult, OP.add)

            ops_ = psM.tile([BC, 128], F32, tag="ps")
            nc.tensor.transpose(ops_[:, :rt], res[:rt, :],
                                ident_f32[0:rt, 0:rt])
            nc.scalar.activation(out_sb[:, r0:r0 + rt], ops_[:, :rt], AF.Copy)

        # all S phases up-front (PE stays hot), then v/qf per tile
        st = [s_phase(*ROW_TILES[i]) for i in range(3)]
        for i in range(3):
            r0, rt = ROW_TILES[i]
            mi, ni = v_phase(r0, rt, *st[i][:3])
            qf_phase(r0, rt, mi, ni, st[i][3], st[i][4])

        # single final output DMA: out_sb [BC, ROWS] -> out_ext [BI, BC, R]
        nc.scalar.dma_start(
            out=out_ext.rearrange("i c r -> c i r"),
            in_=out_sb[:].rearrange("c (i r) -> c i r", r=R))

    nc.finalize()
    return nc


def _get_runner(a: float):
    key = round(float(a), 9)
    if key not in _CACHE:
        _CACHE[key] = _build(key)
    return _CACHE[key]


def _gcaps_padded(capsT, core):
    """[D, PPC*128]: pair j -> even cap words at cols 128j+0:50, odd cap
    words at 128j+64:114, rest zero (32-aligned PSUM block reads)."""
    g = np.zeros((D, GW), dtype=np.float16)
    base = core * CPC * W
    for j in range(PPC):
        g[:, 128 * j:128 * j + 50] = capsT[:, base + 100 * j:
                                           base + 100 * j + 50]
        g[:, 128 * j + 64:128 * j + 114] = capsT[:, base + 100 * j + 50:
                                                 base + 100 * j + 100]
    return g


def _host_prep(imgs, caps, img_lens, cap_lens):
    imgs = np.ascontiguousarray(np.asarray(imgs, dtype=np.float32))
    caps = np.ascontiguousarray(np.asarray(caps, dtype=np.float32))
    img_lens = np.asarray(img_lens).astype(np.int64)
    cap_lens = np.asarray(cap_lens).astype(np.int64)

    capsT = np.ascontiguousarray(
        caps.reshape(BC * W, D).T).astype(np.float16)   # [D, CW]
    cap_mask = (np.arange(W)[:, None] < cap_lens[None, :]).astype(np.float32)
    cm_cw = cap_mask.T.reshape(1, CW)
    adds_row = np.where(cm_cw > 0, 0.0, NEGS).astype(np.float16)
    inv_nc = 1.0 / (np.linalg.norm(caps.astype(np.float64), axis=-1) + EPS)
    scale_row = (inv_nc.reshape(1, CW) * cm_cw
                 + KMASK * (1.0 - cm_cw)).astype(np.float16)
    bc_scale_in = np.ascontiguousarray(
        np.broadcast_to(scale_row, (128, CW)))

    in_maps = []
    for core in range(N_CORES):
        sl = slice(core * BI, (core + 1) * BI)
        im = imgs[sl].reshape(ROWS, D)
        imT = np.ascontiguousarray(im.T).astype(np.float16)
        iv = (np.arange(R)[None, :] < img_lens[sl][:, None]).astype(
            np.float32).reshape(ROWS, 1)
        in_maps.append({
            "capsT": capsT,
            "gcaps": _gcaps_padded(capsT, core),
            "imgsT": imT,
            "imgs_nat": im,
            "bc_scale_in": bc_scale_in,
            "adds_row": adds_row,
            "iv_col": iv,
            "ivm1_col": iv - 1.0,
        })
    return in_maps


def run_on_device(inputs: dict, trace: bool = False):
    """Returns (output [64,64,36] f32, BassKernelResults)."""
    from concourse.bass_utils import run_bass_kernel_spmd
    alpha = float(np.asarray(inputs["alpha"]).reshape(-1)[0])
    a = 1.0 / (1.0 + np.exp(-alpha))
    nc = _get_runner(a)
    in_maps = _host_prep(inputs["imgs"], inputs["caps"], inputs["img_lens"],
                         inputs["cap_lens"])
    r = run_bass_kernel_spmd(nc, in_maps, list(range(N_CORES)), trace=trace)
    out = np.concatenate([r.results[c]["out"][None] for c in range(N_CORES)],
                         axis=0)
    return out.reshape(B, BC, R).astype(np.float32), r


def kernel(imgs, caps, img_lens, cap_lens, alpha):
    out, _ = run_on_device({"imgs": imgs, "caps": caps, "img_lens": img_lens,
                            "cap_lens": cap_lens, "alpha": alpha})
    return out
